# revision 36
# baseline (speedup 1.0000x reference)
"""Causal attention block (q,k,v,mask -> (output, attention)) on 8 trn2 cores.

Sharding: B*H = 32 head-slices split 4-per-core across 8 cores (head
parallel, zero communication). Each core computes, for its 4 heads:
    scores = (q @ k^T) / sqrt(d_key), additive mask, softmax, out = A @ v
and writes the [S,S] attention matrix (fp32) and the [S,D] output.

Structure (heads processed in interleaved pairs so the PE always has
independent work):
  - Q^T/K^T are transposed on the HOST and shipped as fp16, stacked so a
    head pair occupies partition halves (row-packed K=64 matmul pairs).
  - N-side (attention output): scores = QT.T @ KT into PSUM (fp16 matmuls,
    fp32 accumulate); ScalarE exp reads PSUM directly with the 1/sqrt(d)
    scale; the causal mask is applied post-exp as a 0/1 triangle multiply
    on VectorE (every causal diagonal block has the same pattern, and
    exp(x)*0 == 0 matches the reference's exp-underflow zeros exactly);
    VectorE row-sum reduce + reciprocal + in-place normalize (2x mode);
    attention rows are staged in double-buffered half-quarter granules and
    written by one rectangular DMA per half (the runtime pre-zeroes output
    buffers). The generic-mask fallback path instead accumulates
    I.T @ penalty rows into PSUM via matmuls before the exp.
  - T-side (out = A @ v): scores are recomputed TRANSPOSED (KT stationary)
    so no on-chip transposes of A are needed; exp writes unnormalized E^T
    (fp16) straight into the staging buffer, lower-triangle zeroed the
    same post-exp way; out^T accumulates
    V^T-stationary matmuls over E^T into one PSUM bank stacked across the
    head pair (chain order guarded: a start=True clears the whole bank's
    has_written bits); out^T is transposed back through the PE (row-packed
    fp16 identity matmuls) with the per-row reciprocal applied during PSUM
    evacuation.
fp16 is used only where it costs little accuracy (measured vs fp32 CPU
reference: attention ~2.6e-4, out ~5e-4 absmax-relative): q/k for score
matmuls, E^T/V for the out matmul. All softmax math, row sums, and the
attention output stay fp32.
"""

import math
import os

import numpy as np

import concourse.bass as bass
import concourse.tile as tile
from concourse import bacc, mybir
from concourse.bass_utils import run_bass_kernel_spmd

FP32 = mybir.dt.float32
F16 = mybir.dt.float16
AF = mybir.ActivationFunctionType

B, H, S, D = 2, 16, 2048, 64
N_CORES = 8
HEADS_PER_CORE = (B * H) // N_CORES  # 4
NB = S // 128  # 16 q/k blocks per head
NQUARTERS = 4
QBLKS = NB // NQUARTERS  # 4 q-blocks per quarter

# Penalty in raw (pre-scale) score space; must be fp16-representable.
# exp(scale * (qk - 57344)) underflows to exactly 0.0f, matching the
# reference's masked_fill(-1e9) -> softmax -> 0 (also exact underflow).
PENALTY = -57344.0


def _attention_body(tc, outs, ins, causal: bool, scale: float):
    """Two heads processed interleaved per pass: independent work keeps the
    PE array busy (HAM warm) while softmax chains drain on ACT/DVE. mm1 is
    row-packed across the head pair (K=64 each -> both halves of the PE
    array run concurrently); Q/K transposes are col-packed the same way."""
    nc = tc.nc
    qt2_d, kt2_d, v_d = ins["qt2"], ins["kt2"], ins["v16"]
    pen_d, pent_d, id_d = ins["pen"], ins["penT"], ins["ident"]
    att_d, out_d = outs["att"], outs["out"]

    with (
        tc.tile_pool(name="singles", bufs=1) as singles,
        tc.tile_pool(name="qkv", bufs=1) as qkv_pool,
        tc.tile_pool(name="v16", bufs=2) as v16_pool,
        tc.tile_pool(name="qt", bufs=2) as qt_pool,
        tc.tile_pool(name="aq", bufs=2) as aq_pool,
        tc.tile_pool(name="at", bufs=1) as at_pool,
        tc.tile_pool(name="pent", bufs=2) as pen_pool,
        tc.tile_pool(name="sums", bufs=4) as sums_pool,
        tc.tile_pool(name="o", bufs=2) as o_pool,
        tc.tile_pool(name="ps_scores", bufs=1, space="PSUM") as ps_scores,
        tc.tile_pool(name="ps_stT", bufs=3, space="PSUM") as ps_stT,
        tc.tile_pool(name="ps_ot", bufs=1, space="PSUM") as ps_ot,
    ):
        ident = singles.tile([128, 128], FP32)
        nc.gpsimd.dma_start(out=ident, in_=id_d)
        ident16 = singles.tile([128, 128], F16)
        nc.gpsimd.tensor_copy(ident16, ident)
        # two stacked 64x64 identities so each head's out-transpose can
        # stream its identity from its own partition half
        identb = singles.tile([128, 64], F16)
        nc.gpsimd.dma_start(out=identb, in_=ins["identb"])
        # 0/1 triangles for post-exp causal masking on DVE (every causal
        # diagonal block has the same pattern): cols 0:128 tril, 128:256 triu
        tri16 = singles.tile([128, 256], F16)
        nc.gpsimd.dma_start(out=tri16, in_=ins["tri16"])

        for hp in range(HEADS_PER_CORE // 2):
            heads = (2 * hp, 2 * hp + 1)
            # ---- host-pretransposed fp16 Q^T/K^T (stacked head pair) ----
            QT2 = qt_pool.tile([128, S], F16, tag="QT2")
            KT2 = qt_pool.tile([128, S], F16, tag="KT2")
            nc.gpsimd.dma_start(out=QT2, in_=qt2_d[hp])
            nc.gpsimd.dma_start(out=KT2, in_=kt2_d[hp])
            V16 = []
            for x, h in enumerate(heads):
                v16 = v16_pool.tile([128, NB, 64], F16, tag=f"V16{x}")
                nc.gpsimd.dma_start(
                    out=v16, in_=v_d[h].rearrange("(c p) d -> p c d", p=128)
                )
                V16.append(v16)

            for qb in range(NQUARTERS):
                kvmax = (qb * QBLKS + QBLKS) * 128 if causal else S
                kcmax = kvmax // 128  # k-chunks consumed by this quarter
                AT = [
                    at_pool.tile([128, NB, 512], F16, tag=f"AT{x}", name=f"AT{x}")
                    for x in range(2)
                ]
                recq = [
                    sums_pool.tile([128, 4], FP32, tag=f"recq{x}", name=f"recq{x}")
                    for x in range(2)
                ]

                for i in range(QBLKS):
                    qi = qb * QBLKS + i
                    kv = (qi + 1) * 128 if causal else S
                    if i % 2 == 0:
                        # half-quarter attention buffers: smaller granules
                        # release sooner, double-buffered at no extra SBUF
                        Aq = [
                            aq_pool.tile(
                                [128, 2, S], FP32, tag=f"Aq{x}", name=f"Aq{x}"
                            )
                            for x in range(2)
                        ]
                    ih = i % 2

                    if not causal:
                        pent = pen_pool.tile([128, S], F16, tag="pent")
                        nc.gpsimd.dma_start(
                            out=pent, in_=pen_d[qi * 128 : (qi + 1) * 128, :]
                        )

                    # ---- scores for both heads, row-packed matmul pairs ----
                    stiles = [[], []]
                    for c0 in range(0, kv, 1024):
                        w = min(1024, kv - c0)
                        sts = [
                            ps_scores.tile([128, 1024], FP32, tag=f"st{x}", name=f"st{x}")
                            for x in range(2)
                        ]
                        for x in range(2):
                            stiles[x].append((sts[x], c0, w))
                        for n0 in range(0, w, 512):
                            n = min(512, w - n0)
                            for x, pb in ((0, 0), (1, 64)):
                                nc.tensor.matmul(
                                    sts[x][:, n0 : n0 + n],
                                    lhsT=QT2[
                                        pb : pb + 64, qi * 128 : (qi + 1) * 128
                                    ],
                                    rhs=KT2[
                                        pb : pb + 64, c0 + n0 : c0 + n0 + n
                                    ],
                                    start=True,
                                    stop=True,
                                )
                        if not causal:
                            for x in range(2):
                                for n0 in range(0, w, 512):
                                    n = min(512, w - n0)
                                    nc.tensor.matmul(
                                        sts[x][:, n0 : n0 + n],
                                        lhsT=ident16,
                                        rhs=pent[:, c0 + n0 : c0 + n0 + n],
                                        start=False,
                                        stop=True,
                                        skip_group_check=True,
                                    )

                    for x in range(2):
                        # exp from PSUM (unmasked); causal diag zeros applied
                        # post-exp by a 0/1 triangle multiply on DVE, keeping
                        # the penalty matmuls off the PE entirely
                        for t, (st, c0, w) in enumerate(stiles[x]):
                            nc.scalar.activation(
                                out=Aq[x][:, ih, c0 : c0 + w],
                                in_=st[:, 0:w],
                                func=AF.Exp,
                                scale=scale,
                            )
                        if causal:
                            nc.vector.tensor_mul(
                                Aq[x][:, ih, qi * 128 : kv],
                                Aq[x][:, ih, qi * 128 : kv],
                                tri16[:, 0:128],
                            )
                        ssum = sums_pool.tile([128, 1], FP32, tag=f"ssum{x}")
                        nc.vector.reduce_sum(
                            ssum,
                            Aq[x][:, ih, 0:kv],
                            axis=mybir.AxisListType.X,
                        )
                        recip = sums_pool.tile([128, 1], FP32, tag=f"recip{x}")
                        nc.vector.reciprocal(recip, ssum)

                        # store recip for the late out scaling
                        nc.gpsimd.tensor_copy(
                            recq[x][:, i : i + 1], recip
                        )
                        # f32 normalize in place for the attention output
                        nc.vector.tensor_scalar_mul(
                            Aq[x][:, ih, 0:kv], Aq[x][:, ih, 0:kv], recip
                        )

                    if i % 2 == 1:
                        # end of a half: pad one staircase block, then DMA
                        kvh = kv if causal else S
                        for x in range(2):
                            if causal:
                                kv_prev = qi * 128
                                nc.gpsimd.memset(
                                    Aq[x][:, 0, kv_prev:kvh], 0.0
                                )
                            r0 = (qi - 1) * 128
                            att_dst = att_d[
                                heads[x], r0 : r0 + 256, 0:kvh
                            ].rearrange("(c p) k -> p c k", p=128)
                            nc.sync.dma_start(
                                out=att_dst, in_=Aq[x][:, :, 0:kvh]
                            )

                # ---- T-side: recompute scores transposed, exp into AT ----
                for kc in range(kcmax):
                    # valid q-cols within the quarter for this k-chunk
                    cstart = max(0, (kc - qb * QBLKS) * 128) if causal else 0
                    cw = 512 - cstart
                    if not causal:
                        pentT = pen_pool.tile(
                            [128, 512], F16, tag="pentT", name="pentT"
                        )
                        nc.gpsimd.dma_start(
                            out=pentT,
                            in_=pent_d[
                                kc * 128 : (kc + 1) * 128,
                                qb * 512 : (qb + 1) * 512,
                            ],
                        )
                    for x, pb in ((0, 0), (1, 64)):
                        stT = ps_stT.tile([128, 512], FP32, tag="stT", name="stT")
                        nc.tensor.matmul(
                            stT[:, cstart:512],
                            lhsT=KT2[pb : pb + 64, kc * 128 : (kc + 1) * 128],
                            rhs=QT2[
                                pb : pb + 64,
                                qb * 512 + cstart : (qb + 1) * 512,
                            ],
                            start=True,
                            stop=True,
                        )
                        if causal:
                            pass
                        else:
                            nc.tensor.matmul(
                                stT[:, 0:512],
                                lhsT=ident16,
                                rhs=pentT,
                                start=False,
                                stop=True,
                                skip_group_check=True,
                            )
                        nc.scalar.activation(
                            out=AT[x][:, kc, cstart:512],
                            in_=stT[:, cstart:512],
                            func=AF.Exp,
                            scale=scale,
                        )
                        if causal and kc >= qb * QBLKS:
                            # zero E^T below the diagonal (q < k) in place
                            nc.vector.tensor_mul(
                                AT[x][:, kc, cstart : cstart + 128],
                                AT[x][:, kc, cstart : cstart + 128],
                                tri16[:, 128:256],
                            )
                        if causal and cstart > 0:
                            nc.gpsimd.memset(AT[x][:, kc, 0:cstart], 0.0)


                # ---- out^T for both heads stacked into one PSUM bank ----
                # head1's start=True clears the whole bank's has_written
                # bits, so its chain must begin only after head0's chain is
                # fully accumulated (values are unaffected by the clear).
                poT2 = ps_ot.tile([128, 512], FP32, tag="poT2", name="poT2")
                h0_last = None
                for x, pb in ((0, 0), (1, 64)):
                    for kc in range(kcmax):
                        mm = nc.tensor.matmul(
                            poT2[pb : pb + 64, :],
                            lhsT=V16[x][:, kc, :],
                            rhs=AT[x][:, kc, :],
                            start=(kc == 0),
                            stop=(kc == kcmax - 1),
                        )
                        if x == 0 and kc == kcmax - 1:
                            h0_last = mm
                        if x == 1 and kc == 0:
                            tile.add_dep_helper(
                                mm.ins,
                                h0_last.ins,
                                reason="stacked psum: head1 start clears bank bits",
                            )
                oT2 = o_pool.tile([128, 512], F16, tag="oT2")
                nc.vector.tensor_copy(oT2, poT2)
                pouts = [
                    ps_stT.tile([128, 256], FP32, tag="stT", name=f"pout{x}")
                    for x in range(2)
                ]
                for j in range(4):
                    for x, pb in ((0, 0), (1, 64)):
                        # row-packed pair: head0 rows h0, head1 rows h1
                        nc.tensor.matmul(
                            pouts[x][:, j * 64 : (j + 1) * 64],
                            lhsT=oT2[pb : pb + 64, j * 128 : (j + 1) * 128],
                            rhs=identb[pb : pb + 64, :],
                            start=True,
                            stop=True,
                        )
                for x in range(2):
                    ob = o_pool.tile(
                        [128, 4, 64], FP32, tag=f"ob{x}", name=f"ob{x}"
                    )
                    for c in range(4):
                        nc.vector.tensor_scalar_mul(
                            ob[:, c, :],
                            pouts[x][:, c * 64 : (c + 1) * 64],
                            recq[x][:, c : c + 1],
                        )
                    out_dst = out_d[
                        heads[x], qb * 512 : (qb + 1) * 512, :
                    ].rearrange("(c p) d -> p c d", p=128)
                    nc.sync.dma_start(out=out_dst, in_=ob)


def build_program(causal: bool, scale: float):
    nc = bacc.Bacc(
        "TRN2",
        target_bir_lowering=False,
        debug=False,
        enable_asserts=False,
        num_devices=N_CORES,
    )
    hp = HEADS_PER_CORE
    ins = {
        "qt2": nc.dram_tensor(
            "qt2", [hp // 2, 128, S], F16, kind="ExternalInput"
        ).ap(),
        "kt2": nc.dram_tensor(
            "kt2", [hp // 2, 128, S], F16, kind="ExternalInput"
        ).ap(),
        "v16": nc.dram_tensor(
            "v16", [hp, S, D], F16, kind="ExternalInput"
        ).ap(),
        "ident": nc.dram_tensor(
            "ident", [128, 128], FP32, kind="ExternalInput"
        ).ap(),
        "identb": nc.dram_tensor(
            "identb", [128, 64], F16, kind="ExternalInput"
        ).ap(),
        "tri16": nc.dram_tensor(
            "tri16", [128, 256], F16, kind="ExternalInput"
        ).ap(),
    }
    pen_shape = [128, S] if causal else [S, S]
    ins["pen"] = nc.dram_tensor(
        "pen", pen_shape, F16, kind="ExternalInput"
    ).ap()
    ins["penT"] = nc.dram_tensor(
        "penT", pen_shape, F16, kind="ExternalInput"
    ).ap()
    outs = {
        "att": nc.dram_tensor(
            "att", [hp, S, S], FP32, kind="ExternalOutput"
        ).ap(),
        "out": nc.dram_tensor(
            "out", [hp, S, D], FP32, kind="ExternalOutput"
        ).ap(),
    }
    with tile.TileContext(nc) as tc:
        _attention_body(tc, outs, ins, causal=causal, scale=scale)
    nc.compile()
    return nc


def prep_core_inputs(qs, ks, vs, pen, penT):
    """Host-side per-core input prep. qs/ks/vs: [heads, S, D] f32 shards."""
    nh = qs.shape[0]
    qt2 = np.empty((nh // 2, 128, S), dtype=np.float16)
    kt2 = np.empty((nh // 2, 128, S), dtype=np.float16)
    for p in range(nh // 2):
        qt2[p, 0:64] = qs[2 * p].T
        qt2[p, 64:128] = qs[2 * p + 1].T
        kt2[p, 0:64] = ks[2 * p].T
        kt2[p, 64:128] = ks[2 * p + 1].T
    return {
        "qt2": qt2,
        "kt2": kt2,
        "v16": vs.astype(np.float16),
        "pen": pen,
        "penT": penT,
        "ident": np.eye(128, dtype=np.float32),
        "identb": np.vstack(
            [np.eye(64, dtype=np.float16), np.eye(64, dtype=np.float16)]
        ),
        "tri16": np.concatenate(
            [
                np.tril(np.ones((128, 128), dtype=np.float16)),
                np.triu(np.ones((128, 128), dtype=np.float16)),
            ],
            axis=1,
        ),
    }


_PROGRAM_CACHE = {}


def _get_program(causal: bool, scale: float):
    key = (causal, scale)
    if key not in _PROGRAM_CACHE:
        _PROGRAM_CACHE[key] = build_program(causal, scale)
    return _PROGRAM_CACHE[key]


def kernel(q, k, v, mask, d_key, mask_value):
    q = np.ascontiguousarray(np.asarray(q, dtype=np.float32))
    k = np.ascontiguousarray(np.asarray(k, dtype=np.float32))
    v = np.ascontiguousarray(np.asarray(v, dtype=np.float32))
    mask2d = np.asarray(mask).reshape(S, S)
    scale = 1.0 / math.sqrt(float(np.asarray(d_key)))

    causal = bool(
        np.array_equal(mask2d != 0, np.tril(np.ones((S, S), dtype=bool)))
    )

    maskf = (mask2d != 0).astype(np.float32)
    if causal:
        # per-q-block diagonal penalty blocks, stacked along free dim
        pen = np.zeros((128, S), dtype=np.float16)
        penT = np.zeros((128, S), dtype=np.float16)
        for qi in range(NB):
            blk = maskf[qi * 128 : (qi + 1) * 128, qi * 128 : (qi + 1) * 128]
            pb = ((1.0 - blk) * PENALTY).astype(np.float16)
            pen[:, qi * 128 : (qi + 1) * 128] = pb
            penT[:, qi * 128 : (qi + 1) * 128] = pb.T
    else:
        pen = ((1.0 - maskf) * PENALTY).astype(np.float16)
        penT = np.ascontiguousarray(pen.T)

    qr = q.reshape(B * H, S, D)
    kr = k.reshape(B * H, S, D)
    vr = v.reshape(B * H, S, D)

    nc = _get_program(causal, scale)
    in_maps = []
    for c in range(N_CORES):
        sl = slice(c * HEADS_PER_CORE, (c + 1) * HEADS_PER_CORE)
        in_maps.append(prep_core_inputs(qr[sl], kr[sl], vr[sl], pen, penT))

    trace = os.environ.get("KERNEL_TRACE") == "1"
    res = run_bass_kernel_spmd(
        nc, in_maps, core_ids=list(range(N_CORES)), trace=trace
    )
    if trace:
        print(f"HW exec time: {res.exec_time_ns} ns")

    att = np.empty((B * H, S, S), dtype=np.float32)
    out = np.empty((B * H, S, D), dtype=np.float32)
    for c in range(N_CORES):
        sl = slice(c * HEADS_PER_CORE, (c + 1) * HEADS_PER_CORE)
        att[sl] = res.results[c]["att"]
        out[sl] = res.results[c]["out"]
    return out.reshape(B, H, S, D), att.reshape(B, H, S, S)


# revision 38
# speedup vs baseline: 1.1686x; 1.1686x over previous
"""Causal attention block (q,k,v,mask -> (output, attention)) on 8 trn2 cores.

Sharding: B*H = 32 head-slices split 4-per-core across 8 cores (head
parallel, zero communication). Each core computes, for its 4 heads:
    scores = (q @ k^T) / sqrt(d_key), additive mask, softmax, out = A @ v
and writes the [S,S] attention matrix (fp32) and the [S,D] output.

Structure (heads processed in interleaved pairs so the PE always has
independent work):
  - Q^T/K^T are transposed on the HOST and shipped as fp16, stacked so a
    head pair occupies partition halves (row-packed K=64 matmul pairs).
  - N-side (attention output): scores = QT.T @ KT into PSUM (fp16 matmuls,
    fp32 accumulate); ScalarE exp reads PSUM directly with the 1/sqrt(d)
    scale; the causal mask is applied post-exp as a 0/1 triangle multiply
    on VectorE (every causal diagonal block has the same pattern, and
    exp(x)*0 == 0 matches the reference's exp-underflow zeros exactly);
    VectorE row-sum reduce + reciprocal + in-place normalize (2x mode);
    attention rows are staged in double-buffered half-quarter granules and
    written by one rectangular DMA per half (the runtime pre-zeroes output
    buffers). The generic-mask fallback path instead accumulates
    I.T @ penalty rows into PSUM via matmuls before the exp.
  - T-side (out = A @ v): scores are recomputed TRANSPOSED (KT stationary)
    so no on-chip transposes of A are needed; exp writes unnormalized E^T
    (fp16) straight into the staging buffer, lower-triangle zeroed the
    same post-exp way; out^T accumulates
    V^T-stationary matmuls over E^T into one PSUM bank stacked across the
    head pair (chain order guarded: a start=True clears the whole bank's
    has_written bits); out^T is transposed back through the PE (row-packed
    fp16 identity matmuls) with the per-row reciprocal applied during PSUM
    evacuation.
fp16 is used only where it costs little accuracy (measured vs fp32 CPU
reference: attention ~2.6e-4, out ~5e-4 absmax-relative): q/k for score
matmuls, E^T/V for the out matmul. All softmax math, row sums, and the
attention output stay fp32.
"""

import math
import os

import numpy as np

import concourse.bass as bass
import concourse.tile as tile
from concourse import bacc, mybir
from concourse.bass_utils import run_bass_kernel_spmd

FP32 = mybir.dt.float32
F16 = mybir.dt.float16
AF = mybir.ActivationFunctionType

B, H, S, D = 2, 16, 2048, 64
N_CORES = 8
HEADS_PER_CORE = (B * H) // N_CORES  # 4
NB = S // 128  # 16 q/k blocks per head
NQUARTERS = 4
QBLKS = NB // NQUARTERS  # 4 q-blocks per quarter

# Penalty in raw (pre-scale) score space; must be fp16-representable.
# exp(scale * (qk - 57344)) underflows to exactly 0.0f, matching the
# reference's masked_fill(-1e9) -> softmax -> 0 (also exact underflow).
PENALTY = -57344.0


def _attention_body(tc, outs, ins, causal: bool, scale: float):
    """Two heads processed interleaved per pass: independent work keeps the
    PE array busy (HAM warm) while softmax chains drain on ACT/DVE. mm1 is
    row-packed across the head pair (K=64 each -> both halves of the PE
    array run concurrently); Q/K transposes are col-packed the same way."""
    nc = tc.nc
    qt2_d, kt2_d, v_d = ins["qt2"], ins["kt2"], ins["v16"]
    pen_d, pent_d, id_d = ins["pen"], ins["penT"], ins["ident"]
    att_d, out_d = outs["att"], outs["out"]

    with (
        tc.tile_pool(name="singles", bufs=1) as singles,
        tc.tile_pool(name="qkv", bufs=1) as qkv_pool,
        tc.tile_pool(name="v16", bufs=2) as v16_pool,
        tc.tile_pool(name="qt", bufs=2) as qt_pool,
        tc.tile_pool(name="aq", bufs=2) as aq_pool,
        tc.tile_pool(name="at", bufs=1) as at_pool,
        tc.tile_pool(name="pent", bufs=2) as pen_pool,
        tc.tile_pool(name="sums", bufs=4) as sums_pool,
        tc.tile_pool(name="o", bufs=2) as o_pool,
        tc.tile_pool(name="ps_scores", bufs=1, space="PSUM") as ps_scores,
        tc.tile_pool(name="ps_stT", bufs=3, space="PSUM") as ps_stT,
        tc.tile_pool(name="ps_ot", bufs=1, space="PSUM") as ps_ot,
    ):
        ident = singles.tile([128, 128], FP32)
        nc.gpsimd.dma_start(out=ident, in_=id_d)
        ident16 = singles.tile([128, 128], F16)
        nc.gpsimd.tensor_copy(ident16, ident)
        # two stacked 64x64 identities so each head's out-transpose can
        # stream its identity from its own partition half
        identb = singles.tile([128, 64], F16)
        nc.gpsimd.dma_start(out=identb, in_=ins["identb"])
        # 0/1 triangles for post-exp causal masking on DVE (every causal
        # diagonal block has the same pattern): cols 0:128 tril, 128:256 triu
        tri16 = singles.tile([128, 256], F16)
        nc.gpsimd.dma_start(out=tri16, in_=ins["tri16"])

        for hp in range(HEADS_PER_CORE // 2):
            heads = (2 * hp, 2 * hp + 1)
            # ---- host-pretransposed fp16 Q^T/K^T (stacked head pair) ----
            QT2 = qt_pool.tile([128, S], F16, tag="QT2")
            KT2 = qt_pool.tile([128, S], F16, tag="KT2")
            nc.gpsimd.dma_start(out=QT2, in_=qt2_d[hp])
            nc.gpsimd.dma_start(out=KT2, in_=kt2_d[hp])
            V16 = []
            for x, h in enumerate(heads):
                v16 = v16_pool.tile([128, NB, 64], F16, tag=f"V16{x}")
                nc.gpsimd.dma_start(
                    out=v16, in_=v_d[h].rearrange("(c p) d -> p c d", p=128)
                )
                V16.append(v16)

            for qb in range(NQUARTERS):
                kvmax = (qb * QBLKS + QBLKS) * 128 if causal else S
                kcmax = kvmax // 128  # k-chunks consumed by this quarter
                AT = [
                    at_pool.tile([128, NB, 512], F16, tag=f"AT{x}", name=f"AT{x}")
                    for x in range(2)
                ]
                recq = [
                    sums_pool.tile([128, 4], FP32, tag=f"recq{x}", name=f"recq{x}")
                    for x in range(2)
                ]

                for i in range(QBLKS):
                    qi = qb * QBLKS + i
                    kv = (qi + 1) * 128 if causal else S
                    if i % 2 == 0:
                        # half-quarter attention buffers: smaller granules
                        # release sooner, double-buffered at no extra SBUF
                        Aq = [
                            aq_pool.tile(
                                [128, 2, S], FP32, tag=f"Aq{x}", name=f"Aq{x}"
                            )
                            for x in range(2)
                        ]
                    ih = i % 2

                    if not causal:
                        pent = pen_pool.tile([128, S], F16, tag="pent")
                        nc.gpsimd.dma_start(
                            out=pent, in_=pen_d[qi * 128 : (qi + 1) * 128, :]
                        )

                    # ---- scores for both heads, row-packed matmul pairs ----
                    stiles = [[], []]
                    for c0 in range(0, kv, 1024):
                        w = min(1024, kv - c0)
                        sts = [
                            ps_scores.tile([128, 1024], FP32, tag=f"st{x}", name=f"st{x}")
                            for x in range(2)
                        ]
                        for x in range(2):
                            stiles[x].append((sts[x], c0, w))
                        for n0 in range(0, w, 512):
                            n = min(512, w - n0)
                            for x, pb in ((0, 0), (1, 64)):
                                nc.tensor.matmul(
                                    sts[x][:, n0 : n0 + n],
                                    lhsT=QT2[
                                        pb : pb + 64, qi * 128 : (qi + 1) * 128
                                    ],
                                    rhs=KT2[
                                        pb : pb + 64, c0 + n0 : c0 + n0 + n
                                    ],
                                    start=True,
                                    stop=True,
                                )
                        if not causal:
                            for x in range(2):
                                for n0 in range(0, w, 512):
                                    n = min(512, w - n0)
                                    nc.tensor.matmul(
                                        sts[x][:, n0 : n0 + n],
                                        lhsT=ident16,
                                        rhs=pent[:, c0 + n0 : c0 + n0 + n],
                                        start=False,
                                        stop=True,
                                        skip_group_check=True,
                                    )

                    for x in range(2):
                        # exp from PSUM (unmasked); causal diag zeros applied
                        # post-exp by a 0/1 triangle multiply on DVE, keeping
                        # the penalty matmuls off the PE entirely
                        for t, (st, c0, w) in enumerate(stiles[x]):
                            nc.scalar.activation(
                                out=Aq[x][:, ih, c0 : c0 + w],
                                in_=st[:, 0:w],
                                func=AF.Exp,
                                scale=scale,
                            )
                        recip = sums_pool.tile([128, 1], FP32, tag=f"recip{x}")
                        if causal:
                            # mask + row sums fused on DVE: the diag block
                            # gets (E*1)*tril with accum_out; the pre-diag
                            # region an in-place copy with accum_out (2x)
                            parts = sums_pool.tile(
                                [128, 2], FP32, tag=f"parts{x}"
                            )
                            nc.vector.scalar_tensor_tensor(
                                out=Aq[x][:, ih, qi * 128 : kv],
                                in0=Aq[x][:, ih, qi * 128 : kv],
                                scalar=1.0,
                                in1=tri16[:, 0:128],
                                op0=mybir.AluOpType.mult,
                                op1=mybir.AluOpType.mult,
                                accum_out=parts[:, 1:2],
                            )
                            if qi > 0:
                                nc.vector.tensor_scalar(
                                    out=Aq[x][:, ih, 0 : qi * 128],
                                    in0=Aq[x][:, ih, 0 : qi * 128],
                                    scalar1=1.0,
                                    scalar2=0.0,
                                    op0=mybir.AluOpType.mult,
                                    op1=mybir.AluOpType.add,
                                    accum_out=parts[:, 0:1],
                                )
                                ssum = sums_pool.tile(
                                    [128, 1], FP32, tag=f"ssum{x}"
                                )
                                nc.vector.tensor_add(
                                    ssum, parts[:, 0:1], parts[:, 1:2]
                                )
                                nc.vector.reciprocal(recip, ssum)
                            else:
                                nc.vector.reciprocal(recip, parts[:, 1:2])
                        else:
                            ssum = sums_pool.tile(
                                [128, 1], FP32, tag=f"ssum{x}"
                            )
                            nc.vector.reduce_sum(
                                ssum,
                                Aq[x][:, ih, 0:kv],
                                axis=mybir.AxisListType.X,
                            )
                            nc.vector.reciprocal(recip, ssum)

                        # store recip for the late out scaling
                        nc.gpsimd.tensor_copy(
                            recq[x][:, i : i + 1], recip
                        )
                        # f32 normalize in place for the attention output
                        nc.vector.tensor_scalar_mul(
                            Aq[x][:, ih, 0:kv], Aq[x][:, ih, 0:kv], recip
                        )

                    if i % 2 == 1:
                        # end of a half: pad one staircase block, then DMA
                        kvh = kv if causal else S
                        for x in range(2):
                            if causal:
                                kv_prev = qi * 128
                                nc.gpsimd.memset(
                                    Aq[x][:, 0, kv_prev:kvh], 0.0
                                )
                            r0 = (qi - 1) * 128
                            att_dst = att_d[
                                heads[x], r0 : r0 + 256, 0:kvh
                            ].rearrange("(c p) k -> p c k", p=128)
                            nc.sync.dma_start(
                                out=att_dst, in_=Aq[x][:, :, 0:kvh]
                            )

                # ---- T-side: recompute scores transposed, exp into AT ----
                for kc in range(kcmax):
                    # valid q-cols within the quarter for this k-chunk
                    cstart = max(0, (kc - qb * QBLKS) * 128) if causal else 0
                    cw = 512 - cstart
                    if not causal:
                        pentT = pen_pool.tile(
                            [128, 512], F16, tag="pentT", name="pentT"
                        )
                        nc.gpsimd.dma_start(
                            out=pentT,
                            in_=pent_d[
                                kc * 128 : (kc + 1) * 128,
                                qb * 512 : (qb + 1) * 512,
                            ],
                        )
                    for x, pb in ((0, 0), (1, 64)):
                        stT = ps_stT.tile([128, 512], FP32, tag="stT", name="stT")
                        nc.tensor.matmul(
                            stT[:, cstart:512],
                            lhsT=KT2[pb : pb + 64, kc * 128 : (kc + 1) * 128],
                            rhs=QT2[
                                pb : pb + 64,
                                qb * 512 + cstart : (qb + 1) * 512,
                            ],
                            start=True,
                            stop=True,
                        )
                        if causal:
                            pass
                        else:
                            nc.tensor.matmul(
                                stT[:, 0:512],
                                lhsT=ident16,
                                rhs=pentT,
                                start=False,
                                stop=True,
                                skip_group_check=True,
                            )
                        nc.scalar.activation(
                            out=AT[x][:, kc, cstart:512],
                            in_=stT[:, cstart:512],
                            func=AF.Exp,
                            scale=scale,
                        )
                        if causal and kc >= qb * QBLKS:
                            # zero E^T below the diagonal (q < k) in place
                            nc.vector.tensor_mul(
                                AT[x][:, kc, cstart : cstart + 128],
                                AT[x][:, kc, cstart : cstart + 128],
                                tri16[:, 128:256],
                            )
                        if causal and cstart > 0:
                            nc.gpsimd.memset(AT[x][:, kc, 0:cstart], 0.0)


                # ---- out^T for both heads stacked into one PSUM bank ----
                # head1's start=True clears the whole bank's has_written
                # bits, so its chain must begin only after head0's chain is
                # fully accumulated (values are unaffected by the clear).
                poT2 = ps_ot.tile([128, 512], FP32, tag="poT2", name="poT2")
                h0_last = None
                for x, pb in ((0, 0), (1, 64)):
                    for kc in range(kcmax):
                        mm = nc.tensor.matmul(
                            poT2[pb : pb + 64, :],
                            lhsT=V16[x][:, kc, :],
                            rhs=AT[x][:, kc, :],
                            start=(kc == 0),
                            stop=(kc == kcmax - 1),
                        )
                        if x == 0 and kc == kcmax - 1:
                            h0_last = mm
                        if x == 1 and kc == 0:
                            tile.add_dep_helper(
                                mm.ins,
                                h0_last.ins,
                                reason="stacked psum: head1 start clears bank bits",
                            )
                oT2 = o_pool.tile([128, 512], F16, tag="oT2")
                nc.vector.tensor_copy(oT2, poT2)
                pouts = [
                    ps_stT.tile([128, 256], FP32, tag="stT", name=f"pout{x}")
                    for x in range(2)
                ]
                for j in range(4):
                    for x, pb in ((0, 0), (1, 64)):
                        # row-packed pair: head0 rows h0, head1 rows h1
                        nc.tensor.matmul(
                            pouts[x][:, j * 64 : (j + 1) * 64],
                            lhsT=oT2[pb : pb + 64, j * 128 : (j + 1) * 128],
                            rhs=identb[pb : pb + 64, :],
                            start=True,
                            stop=True,
                        )
                for x in range(2):
                    ob = o_pool.tile(
                        [128, 4, 64], FP32, tag=f"ob{x}", name=f"ob{x}"
                    )
                    for c in range(4):
                        nc.vector.tensor_scalar_mul(
                            ob[:, c, :],
                            pouts[x][:, c * 64 : (c + 1) * 64],
                            recq[x][:, c : c + 1],
                        )
                    out_dst = out_d[
                        heads[x], qb * 512 : (qb + 1) * 512, :
                    ].rearrange("(c p) d -> p c d", p=128)
                    nc.sync.dma_start(out=out_dst, in_=ob)


def build_program(causal: bool, scale: float):
    nc = bacc.Bacc(
        "TRN2",
        target_bir_lowering=False,
        debug=False,
        enable_asserts=False,
        num_devices=N_CORES,
    )
    hp = HEADS_PER_CORE
    ins = {
        "qt2": nc.dram_tensor(
            "qt2", [hp // 2, 128, S], F16, kind="ExternalInput"
        ).ap(),
        "kt2": nc.dram_tensor(
            "kt2", [hp // 2, 128, S], F16, kind="ExternalInput"
        ).ap(),
        "v16": nc.dram_tensor(
            "v16", [hp, S, D], F16, kind="ExternalInput"
        ).ap(),
        "ident": nc.dram_tensor(
            "ident", [128, 128], FP32, kind="ExternalInput"
        ).ap(),
        "identb": nc.dram_tensor(
            "identb", [128, 64], F16, kind="ExternalInput"
        ).ap(),
        "tri16": nc.dram_tensor(
            "tri16", [128, 256], F16, kind="ExternalInput"
        ).ap(),
    }
    pen_shape = [128, S] if causal else [S, S]
    ins["pen"] = nc.dram_tensor(
        "pen", pen_shape, F16, kind="ExternalInput"
    ).ap()
    ins["penT"] = nc.dram_tensor(
        "penT", pen_shape, F16, kind="ExternalInput"
    ).ap()
    outs = {
        "att": nc.dram_tensor(
            "att", [hp, S, S], FP32, kind="ExternalOutput"
        ).ap(),
        "out": nc.dram_tensor(
            "out", [hp, S, D], FP32, kind="ExternalOutput"
        ).ap(),
    }
    with tile.TileContext(nc) as tc:
        _attention_body(tc, outs, ins, causal=causal, scale=scale)
    nc.compile()
    return nc


def prep_core_inputs(qs, ks, vs, pen, penT):
    """Host-side per-core input prep. qs/ks/vs: [heads, S, D] f32 shards."""
    nh = qs.shape[0]
    qt2 = np.empty((nh // 2, 128, S), dtype=np.float16)
    kt2 = np.empty((nh // 2, 128, S), dtype=np.float16)
    for p in range(nh // 2):
        qt2[p, 0:64] = qs[2 * p].T
        qt2[p, 64:128] = qs[2 * p + 1].T
        kt2[p, 0:64] = ks[2 * p].T
        kt2[p, 64:128] = ks[2 * p + 1].T
    return {
        "qt2": qt2,
        "kt2": kt2,
        "v16": vs.astype(np.float16),
        "pen": pen,
        "penT": penT,
        "ident": np.eye(128, dtype=np.float32),
        "identb": np.vstack(
            [np.eye(64, dtype=np.float16), np.eye(64, dtype=np.float16)]
        ),
        "tri16": np.concatenate(
            [
                np.tril(np.ones((128, 128), dtype=np.float16)),
                np.triu(np.ones((128, 128), dtype=np.float16)),
            ],
            axis=1,
        ),
    }


_PROGRAM_CACHE = {}


def _get_program(causal: bool, scale: float):
    key = (causal, scale)
    if key not in _PROGRAM_CACHE:
        _PROGRAM_CACHE[key] = build_program(causal, scale)
    return _PROGRAM_CACHE[key]


def kernel(q, k, v, mask, d_key, mask_value):
    q = np.ascontiguousarray(np.asarray(q, dtype=np.float32))
    k = np.ascontiguousarray(np.asarray(k, dtype=np.float32))
    v = np.ascontiguousarray(np.asarray(v, dtype=np.float32))
    mask2d = np.asarray(mask).reshape(S, S)
    scale = 1.0 / math.sqrt(float(np.asarray(d_key)))

    causal = bool(
        np.array_equal(mask2d != 0, np.tril(np.ones((S, S), dtype=bool)))
    )

    maskf = (mask2d != 0).astype(np.float32)
    if causal:
        # per-q-block diagonal penalty blocks, stacked along free dim
        pen = np.zeros((128, S), dtype=np.float16)
        penT = np.zeros((128, S), dtype=np.float16)
        for qi in range(NB):
            blk = maskf[qi * 128 : (qi + 1) * 128, qi * 128 : (qi + 1) * 128]
            pb = ((1.0 - blk) * PENALTY).astype(np.float16)
            pen[:, qi * 128 : (qi + 1) * 128] = pb
            penT[:, qi * 128 : (qi + 1) * 128] = pb.T
    else:
        pen = ((1.0 - maskf) * PENALTY).astype(np.float16)
        penT = np.ascontiguousarray(pen.T)

    qr = q.reshape(B * H, S, D)
    kr = k.reshape(B * H, S, D)
    vr = v.reshape(B * H, S, D)

    nc = _get_program(causal, scale)
    in_maps = []
    for c in range(N_CORES):
        sl = slice(c * HEADS_PER_CORE, (c + 1) * HEADS_PER_CORE)
        in_maps.append(prep_core_inputs(qr[sl], kr[sl], vr[sl], pen, penT))

    trace = os.environ.get("KERNEL_TRACE") == "1"
    res = run_bass_kernel_spmd(
        nc, in_maps, core_ids=list(range(N_CORES)), trace=trace
    )
    if trace:
        print(f"HW exec time: {res.exec_time_ns} ns")

    att = np.empty((B * H, S, S), dtype=np.float32)
    out = np.empty((B * H, S, D), dtype=np.float32)
    for c in range(N_CORES):
        sl = slice(c * HEADS_PER_CORE, (c + 1) * HEADS_PER_CORE)
        att[sl] = res.results[c]["att"]
        out[sl] = res.results[c]["out"]
    return out.reshape(B, H, S, D), att.reshape(B, H, S, S)
